# revision 35
# baseline (speedup 1.0000x reference)
"""Trainium2 Bass kernel for nn_MultiHeadAttention_76510547410991.

The reference's reshapes apply identically to both factors of the
elementwise product, so they cancel and the computation is exactly:
    out = ((x @ Wq.T + bq) * (value @ Wv.T + bv)) @ Wc.T + bc

Sharding: rows (S=32768) split across 8 cores, 4096 rows each; weights
replicated.  All activations are kept in the transposed (feature-major)
domain on-chip so that neither the Q/V projections nor the final
C-projection need any on-device transposes; the host pre-transposes the
inputs (cheap numpy copies, outside the device clock).

All matmuls run in float32r (TF32-like PE fast path, 4x the fp32 rate;
measured end-to-end relative error ~2.5e-4 vs fp64).

Per-core dataflow, row-tile RT=512:
  xT,vT [256, 4096]   (host-transposed shards)
  for each row-tile n:
    for m in 16 feature slices of 2048:
      qp[128,512](PSUM)  = WqT_k-slices.T @ xT_k        (2 matmuls, f32r)
      vp[128,512](PSUM)  = WvT_k-slices.T @ vT_k
      qb = ACT(qp + bq_m)  (PSUM->SBUF, per-partition bias fused)
      vb = ACT(vp + bv_m)
      pT_m = DVE qb*vb     (SBUF, f32r)
    for s in 4 row-subtiles of 128:
      op[128,256](PSUM) = sum_m pT_m_s.T @ WcT_m        (16 matmuls, f32r)
      out_s = DVE (op + bc)                              (bias broadcast)
      DMA out_s -> out[rows, 256]
Startup DMAs are ordered x0/bq/Wq-piece/v0/bv/Wv-piece/... so the first
matmul fires after ~1MB of DMA instead of after all 6MB of weights.
PSUM banks are split 3 (q) / 2 (v) / 3 (c-proj out) — HW-A/B-tuned.

Measured (8 axon-tunneled trn2 cores, steady state via in-NEFF-reps
A/B deltas): ~145-157 us per invocation, against a ~132 us pure-matmul
floor at the observed PE rate (512 q/v matmuls alone measure ~88 us;
f32r runs ~173 ns per 512-row matmul on HW).  Engine budget per core:
PE ~132 us, ACT 256 bias ops ~130-158 us, DVE ~89 us, DMA 18 MB ~53 us.
"""

import numpy as np

try:
    import concourse.bacc  # noqa: F401
except ImportError:  # fresh environment without the default sys.path setup
    import sys

    for p in ("/opt/trn_rl_repo", "/opt/pypackages"):
        if p not in sys.path:
            sys.path.insert(0, p)

H = 8
F = 256
S = 32768
FH = F * H  # 2048
D = F  # output features 256
N_CORES = 8
R = S // N_CORES  # 4096 rows per core
RT = 512  # rows per row-tile (fp32 moving-operand max)
NT = R // RT  # 8 row tiles per core
NM = FH // 128  # 16 feature slices
NK = F // 128  # 2 contraction slices for q/v proj

_CACHE = {}


def build_program(
    reps=1,
    mm_mode="f32r",
    qpsum_bufs=3,
    vpsum_bufs=2,
    opsum_bufs=3,
    qv_bufs=4,
    xv_bufs=3,
    pt_bufs=2,
    o_bufs=4,
    rt=RT,
    dve_bias_per_tile=0,  # 0..2*NM: how many of the bias ops go to DVE
    mul_on_pool=0,  # 0..NM: how many of the per-m muls go to GPSIMD
    merge_sp=False,  # merge c-proj 128-row subtile pairs into one PSUM bank
    probe=None,  # "pe_only" | "pe_cp" | "no_act" — timing-only diagnostics
    taper=False,  # 256-row first/last tiles (sim: net loss, keep off)
    qvp_bf16=False,  # qb/vb/pt (and Wc) in bf16: 2x DVE mul, bf16 c-proj
    compile=True,
):
    """Build + compile the per-core Bass program (identical on all cores)."""
    import concourse.bacc as bacc
    import concourse.mybir as mybir
    import concourse.tile as tile

    f32 = mybir.dt.float32
    if mm_mode == "f32r":
        msd = mybir.dt.float32r  # storage dtype for matmul operands
    elif mm_mode == "f32":
        msd = f32
    else:
        raise ValueError(mm_mode)
    bf16 = mybir.dt.bfloat16
    ew_dt = bf16 if qvp_bf16 else f32  # qb/vb dtype
    pt_dt = bf16 if qvp_bf16 else msd  # pt / wc dtype (c-proj operands)

    nc = bacc.Bacc(
        "TRN2",
        target_bir_lowering=False,
        debug=False,
        enable_asserts=False,
        num_devices=N_CORES,
    )

    x_d = nc.dram_tensor("xT", [F, R], msd, kind="ExternalInput").ap()
    v_d = nc.dram_tensor("vT", [F, R], msd, kind="ExternalInput").ap()
    wq_d = nc.dram_tensor("wqT", [F, FH], msd, kind="ExternalInput").ap()
    wv_d = nc.dram_tensor("wvT", [F, FH], msd, kind="ExternalInput").ap()
    wc_d = nc.dram_tensor("wcT", [FH, D], pt_dt, kind="ExternalInput").ap()
    bq_d = nc.dram_tensor("bq2", [128, NM], f32, kind="ExternalInput").ap()
    bv_d = nc.dram_tensor("bv2", [128, NM], f32, kind="ExternalInput").ap()
    bc_d = nc.dram_tensor("bcb", [128, 2 * D], f32, kind="ExternalInput").ap()
    out_d = nc.dram_tensor("out", [R, D], f32, kind="ExternalOutput").ap()

    Act_Id = mybir.ActivationFunctionType.Identity

    if taper:
        # small first tile -> first matmuls fire after ~0.7MB of DMA;
        # small last tile -> shorter final dependency chain.
        schedule = [256] + [rt] * ((R - 512) // rt) + [256]
    else:
        schedule = [rt] * (R // rt)
    assert sum(schedule) == R
    starts = [sum(schedule[:i]) for i in range(len(schedule))]

    def mm_chunks(rtn):
        # moving-dim chunks of <=512 (f32r needs >=256 for full rate)
        return [slice(h, min(h + 512, rtn)) for h in range(0, rtn, 512)]

    with tile.TileContext(nc) as tc:
        with (
            tc.tile_pool(name="w", bufs=1) as wpool,
            tc.tile_pool(name="xv", bufs=xv_bufs) as xvpool,
            tc.tile_pool(name="qv", bufs=qv_bufs) as qvpool,
            tc.tile_pool(name="p", bufs=pt_bufs) as ppool,
            tc.tile_pool(name="o", bufs=o_bufs) as opool,
            tc.tile_pool(name="qpsum", bufs=qpsum_bufs, space="PSUM") as qpsum,
            tc.tile_pool(
                name="vpsum",
                bufs=vpsum_bufs if vpsum_bufs is not None else qpsum_bufs,
                space="PSUM",
            ) as vpsum,
            tc.tile_pool(name="opsum", bufs=opsum_bufs, space="PSUM") as opsum,
        ):
            def load_one(pool_tag, dram, n, k):
                r0, rtn = starts[n], schedule[n]
                t = xvpool.tile([128, rtn], msd, tag=f"{pool_tag}{k}")
                nc.sync.dma_start(
                    t[:], dram[k * 128 : (k + 1) * 128, r0 : r0 + rtn]
                )
                return t

            def load_xv(n):
                xt = [load_one("x", x_d, n, k) for k in range(NK)]
                vt = [load_one("v", v_d, n, k) for k in range(NK)]
                return xt, vt

            # Startup order: x tile + q bias + first q-weight piece first so
            # the first matmul fires after ~0.75MB of DMA, then v-side, then
            # the rest of the weights.
            x0 = [load_one("x", x_d, 0, k) for k in range(NK)]
            bq_sb = wpool.tile([128, NM], f32, tag="bq")
            nc.sync.dma_start(bq_sb[:], bq_d[:, :])

            NQ = 4
            qw = FH // NQ  # 512 columns per piece
            wq_sb = [[None] * NQ for _ in range(NK)]
            wv_sb = [[None] * NQ for _ in range(NK)]

            def load_w(dst, dram, q, k, nm):
                qs = slice(q * qw, (q + 1) * qw)
                t = wpool.tile([128, qw], msd, tag=f"{nm}{k}q{q}")
                nc.sync.dma_start(t[:], dram[k * 128 : (k + 1) * 128, qs])
                dst[k][q] = t

            for k in range(NK):
                load_w(wq_sb, wq_d, 0, k, "wq")
            v0 = [load_one("v", v_d, 0, k) for k in range(NK)]
            bv_sb = wpool.tile([128, NM], f32, tag="bv")
            nc.sync.dma_start(bv_sb[:], bv_d[:, :])
            for k in range(NK):
                load_w(wv_sb, wv_d, 0, k, "wv")
            xv0 = (x0, v0)
            for q in range(1, NQ):
                for k in range(NK):
                    load_w(wq_sb, wq_d, q, k, "wq")
                for k in range(NK):
                    load_w(wv_sb, wv_d, q, k, "wv")
            bc_sb = wpool.tile([128, 2 * D], f32, tag="bc")
            nc.sync.dma_start(bc_sb[:], bc_d[:, :])

            mpq = qw // 128  # m-slices per piece

            def wq_ap(k, m):
                return wq_sb[k][m // mpq][:, (m % mpq) * 128 : (m % mpq + 1) * 128]

            def wv_ap(k, m):
                return wv_sb[k][m // mpq][:, (m % mpq) * 128 : (m % mpq + 1) * 128]
            wc_sb = []
            for m in range(NM):
                t = wpool.tile([128, D], pt_dt, tag=f"wc{m}")
                nc.sync.dma_start(t[:], wc_d[m * 128 : (m + 1) * 128, :])
                wc_sb.append(t)

            for rep in range(reps):
                for n in range(len(schedule)):
                    r0, rtn = starts[n], schedule[n]
                    if rep == 0 and n == 0:
                        xt, vt = xv0
                    else:
                        xt, vt = load_xv(n)

                    pt = ppool.tile([128, NM * rtn], pt_dt, tag="pt")
                    for m in range(NM):
                        qp = qpsum.tile([128, rtn], f32, tag="qp")
                        for hs in mm_chunks(rtn):
                            for k in range(NK):
                                nc.tensor.matmul(
                                    qp[:, hs],
                                    wq_ap(k, m),
                                    xt[k][:, hs],
                                    start=(k == 0),
                                    stop=(k == NK - 1),
                                )
                        vp = vpsum.tile([128, rtn], f32, tag="vp")
                        for hs in mm_chunks(rtn):
                            for k in range(NK):
                                nc.tensor.matmul(
                                    vp[:, hs],
                                    wv_ap(k, m),
                                    vt[k][:, hs],
                                    start=(k == 0),
                                    stop=(k == NK - 1),
                                )
                        if probe == "pe_only":
                            continue
                        if probe == "pe_cp":
                            # timing probe: pt via cheap DVE copy, no ACT
                            nc.vector.tensor_copy(
                                pt[:, m * rtn : (m + 1) * rtn], qp[:]
                            )
                            continue
                        if probe == "no_act":
                            # timing probe: multiply straight from both PSUMs
                            nc.vector.tensor_mul(
                                pt[:, m * rtn : (m + 1) * rtn], qp[:], vp[:]
                            )
                            continue
                        qb = qvpool.tile([128, rtn], ew_dt, tag="qb")
                        if 2 * m + 1 < dve_bias_per_tile:
                            nc.vector.tensor_scalar_add(
                                qb[:], qp[:], bq_sb[:, m : m + 1]
                            )
                        else:
                            nc.scalar.activation(
                                qb[:], qp[:], Act_Id, bias=bq_sb[:, m : m + 1]
                            )
                        vb = qvpool.tile([128, rtn], ew_dt, tag="vb")
                        if 2 * m < dve_bias_per_tile:
                            nc.vector.tensor_scalar_add(
                                vb[:], vp[:], bv_sb[:, m : m + 1]
                            )
                        else:
                            nc.scalar.activation(
                                vb[:], vp[:], Act_Id, bias=bv_sb[:, m : m + 1]
                            )
                        mul_eng = nc.gpsimd if m < mul_on_pool else nc.vector
                        mul_eng.tensor_mul(
                            pt[:, m * rtn : (m + 1) * rtn], qb[:], vb[:]
                        )

                    if probe == "pe_only":
                        continue  # q/v matmuls only
                    if merge_sp:
                        for sp in range(rtn // 256):
                            # two 128-row c-proj groups share one PSUM bank;
                            # one bias-add + one (rearranged) store for both
                            op = opsum.tile([128, 2 * D], f32, tag="op")
                            for half in range(2):
                                s = 2 * sp + half
                                oslice = slice(half * D, (half + 1) * D)
                                for m in range(NM):
                                    c0 = m * rtn + s * 128
                                    nc.tensor.matmul(
                                        op[:, oslice],
                                        pt[:, c0 : c0 + 128],
                                        wc_sb[m][:],
                                        start=(m == 0),
                                        stop=(m == NM - 1),
                                    )
                            ot = opool.tile([128, 2 * D], f32, tag="ot")
                            nc.vector.tensor_add(ot[:], op[:], bc_sb[:])
                            dst = out_d[
                                r0 + sp * 256 : r0 + (sp + 1) * 256, :
                            ].rearrange("(two p) c -> p two c", two=2)
                            nc.sync.dma_start(
                                dst,
                                ot[:].rearrange("p (two c) -> p two c", two=2),
                            )
                    else:
                        for s in range(rtn // 128):
                            op = opsum.tile([128, D], f32, tag="op")
                            for m in range(NM):
                                c0 = m * rtn + s * 128
                                nc.tensor.matmul(
                                    op[:],
                                    pt[:, c0 : c0 + 128],
                                    wc_sb[m][:],
                                    start=(m == 0),
                                    stop=(m == NM - 1),
                                )
                            ot = opool.tile([128, D], f32, tag="ot")
                            nc.vector.tensor_add(ot[:], op[:], bc_sb[:, :D])
                            nc.sync.dma_start(
                                out_d[r0 + s * 128 : r0 + (s + 1) * 128, :],
                                ot[:],
                            )

    if compile:
        nc.compile()
    return nc


def prep_in_maps(query_key_input, value, Wq, bq, Wv, bv, Wc, bc, qvp_bf16=False):
    """Host-side shard + layout prep. Returns list of 8 per-core input dicts."""
    if qvp_bf16:
        import ml_dtypes

        wc_np = ml_dtypes.bfloat16
    else:
        wc_np = np.float32
    x = np.asarray(query_key_input, dtype=np.float32)
    v = np.asarray(value, dtype=np.float32)
    shared = {
        "wqT": np.ascontiguousarray(np.asarray(Wq, np.float32).T),
        "wvT": np.ascontiguousarray(np.asarray(Wv, np.float32).T),
        "wcT": np.ascontiguousarray(np.asarray(Wc, np.float32).T.astype(wc_np)),
        "bq2": np.ascontiguousarray(np.asarray(bq, np.float32).reshape(NM, 128).T),
        "bv2": np.ascontiguousarray(np.asarray(bv, np.float32).reshape(NM, 128).T),
        "bcb": np.ascontiguousarray(
            np.broadcast_to(
                np.tile(np.asarray(bc, np.float32), 2), (128, 2 * D)
            )
        ),
    }
    in_maps = []
    for c in range(N_CORES):
        rows = slice(c * R, (c + 1) * R)
        m = dict(shared)
        m["xT"] = np.ascontiguousarray(x[rows].T)
        m["vT"] = np.ascontiguousarray(v[rows].T)
        in_maps.append(m)
    return in_maps


def run_program(nc, in_maps):
    from concourse import bass_utils

    res = bass_utils.run_bass_kernel_spmd(nc, in_maps, core_ids=list(range(N_CORES)))
    return res


def kernel(query_key_input, value, Wq, bq, Wk, bk, Wv, bv, Wc, bc):
    if "nc" not in _CACHE:
        _CACHE["nc"] = build_program(reps=1)
    nc = _CACHE["nc"]
    in_maps = prep_in_maps(query_key_input, value, Wq, bq, Wv, bv, Wc, bc)
    res = run_program(nc, in_maps)
    out = np.concatenate([res.results[c]["out"] for c in range(N_CORES)], axis=0)
    return out


# revision 36
# speedup vs baseline: 1.1093x; 1.1093x over previous
"""Trainium2 Bass kernel for nn_MultiHeadAttention_76510547410991.

The reference's reshapes apply identically to both factors of the
elementwise product, so they cancel and the computation is exactly:
    out = ((x @ Wq.T + bq) * (value @ Wv.T + bv)) @ Wc.T + bc

Sharding: rows (S=32768) split across 8 cores, 4096 rows each; weights
replicated.  All activations are kept in the transposed (feature-major)
domain on-chip so that neither the Q/V projections nor the final
C-projection need any on-device transposes; the host pre-transposes the
inputs (cheap numpy copies, outside the device clock).

All matmuls run in float32r (TF32-like PE fast path, 4x the fp32 rate;
measured end-to-end relative error ~2.5e-4 vs fp64).

Per-core dataflow, row-tile RT=512:
  xT,vT [256, 4096]   (host-transposed shards)
  for each row-tile n:
    for m in 16 feature slices of 2048:
      qp[128,512](PSUM)  = WqT_k-slices.T @ xT_k        (2 matmuls, f32r)
      vp[128,512](PSUM)  = WvT_k-slices.T @ vT_k
      qb = ACT(qp + bq_m)  (PSUM->SBUF, per-partition bias fused)
      vb = ACT(vp + bv_m)
      pT_m = DVE qb*vb     (SBUF, f32r)
    for s in 4 row-subtiles of 128:
      op[128,256](PSUM) = sum_m pT_m_s.T @ WcT_m        (16 matmuls, f32r)
      out_s = DVE (op + bc)                              (bias broadcast)
      DMA out_s -> out[rows, 256]
Startup DMAs are ordered x0/bq/Wq-piece/v0/bv/Wv-piece/... so the first
matmul fires after ~1MB of DMA instead of after all 6MB of weights.
PSUM banks are split 3 (q) / 2 (v) / 3 (c-proj out) — HW-A/B-tuned.

Measured (8 axon-tunneled trn2 cores, steady state via in-NEFF-reps
A/B deltas): ~145-157 us per invocation, against a ~132 us pure-matmul
floor at the observed PE rate (512 q/v matmuls alone measure ~88 us;
f32r runs ~173 ns per 512-row matmul on HW).  Engine budget per core:
PE ~132 us, ACT 256 bias ops ~130-158 us, DVE ~89 us, DMA 18 MB ~53 us.
"""

import numpy as np

try:
    import concourse.bacc  # noqa: F401
except ImportError:  # fresh environment without the default sys.path setup
    import sys

    for p in ("/opt/trn_rl_repo", "/opt/pypackages"):
        if p not in sys.path:
            sys.path.insert(0, p)

H = 8
F = 256
S = 32768
FH = F * H  # 2048
D = F  # output features 256
N_CORES = 8
R = S // N_CORES  # 4096 rows per core
RT = 512  # rows per row-tile (fp32 moving-operand max)
NT = R // RT  # 8 row tiles per core
NM = FH // 128  # 16 feature slices
NK = F // 128  # 2 contraction slices for q/v proj

_CACHE = {}


def build_program(
    reps=1,
    mm_mode="f32r",
    qpsum_bufs=3,
    vpsum_bufs=2,
    opsum_bufs=3,
    qv_bufs=4,
    xv_bufs=3,
    pt_bufs=2,
    o_bufs=4,
    rt=RT,
    dve_bias_per_tile=0,  # 0..2*NM: how many of the bias ops go to DVE
    mul_on_pool=0,  # 0..NM: how many of the per-m muls go to GPSIMD
    merge_sp=False,  # merge c-proj 128-row subtile pairs into one PSUM bank
    probe=None,  # "pe_only" | "pe_cp" | "no_act" — timing-only diagnostics
    taper=False,  # 256-row first/last tiles (sim: net loss, keep off)
    qvp_bf16=False,  # qb/vb/pt (and Wc) in bf16: 2x DVE mul, bf16 c-proj
    compile=True,
):
    """Build + compile the per-core Bass program (identical on all cores)."""
    import concourse.bacc as bacc
    import concourse.mybir as mybir
    import concourse.tile as tile

    f32 = mybir.dt.float32
    if mm_mode == "f32r":
        msd = mybir.dt.float32r  # storage dtype for matmul operands
    elif mm_mode == "f32":
        msd = f32
    else:
        raise ValueError(mm_mode)
    bf16 = mybir.dt.bfloat16
    ew_dt = bf16 if qvp_bf16 else f32  # qb/vb dtype
    pt_dt = bf16 if qvp_bf16 else msd  # pt / wc dtype (c-proj operands)

    nc = bacc.Bacc(
        "TRN2",
        target_bir_lowering=False,
        debug=False,
        enable_asserts=False,
        num_devices=N_CORES,
    )

    x_d = nc.dram_tensor("xT", [F, R], msd, kind="ExternalInput").ap()
    v_d = nc.dram_tensor("vT", [F, R], msd, kind="ExternalInput").ap()
    wq_d = nc.dram_tensor("wqT", [F, FH], msd, kind="ExternalInput").ap()
    wv_d = nc.dram_tensor("wvT", [F, FH], msd, kind="ExternalInput").ap()
    wc_d = nc.dram_tensor("wcT", [FH, D], pt_dt, kind="ExternalInput").ap()
    bq_d = nc.dram_tensor("bq2", [128, NM], f32, kind="ExternalInput").ap()
    bv_d = nc.dram_tensor("bv2", [128, NM], f32, kind="ExternalInput").ap()
    bc_d = nc.dram_tensor("bcb", [128, 2 * D], f32, kind="ExternalInput").ap()
    out_d = nc.dram_tensor("out", [R, D], f32, kind="ExternalOutput").ap()

    Act_Id = mybir.ActivationFunctionType.Identity

    if taper:
        # small first tile -> first matmuls fire after ~0.7MB of DMA;
        # small last tile -> shorter final dependency chain.
        schedule = [256] + [rt] * ((R - 512) // rt) + [256]
    else:
        schedule = [rt] * (R // rt)
    assert sum(schedule) == R
    starts = [sum(schedule[:i]) for i in range(len(schedule))]

    def mm_chunks(rtn):
        # moving-dim chunks of <=512 (f32r needs >=256 for full rate)
        return [slice(h, min(h + 512, rtn)) for h in range(0, rtn, 512)]

    with tile.TileContext(nc) as tc:
        with (
            tc.tile_pool(name="w", bufs=1) as wpool,
            tc.tile_pool(name="xv", bufs=xv_bufs) as xvpool,
            tc.tile_pool(name="qv", bufs=qv_bufs) as qvpool,
            tc.tile_pool(name="p", bufs=pt_bufs) as ppool,
            tc.tile_pool(name="o", bufs=o_bufs) as opool,
            tc.tile_pool(name="qpsum", bufs=qpsum_bufs, space="PSUM") as qpsum,
            tc.tile_pool(
                name="vpsum",
                bufs=vpsum_bufs if vpsum_bufs is not None else qpsum_bufs,
                space="PSUM",
            ) as vpsum,
            tc.tile_pool(name="opsum", bufs=opsum_bufs, space="PSUM") as opsum,
        ):
            def load_one(pool_tag, dram, n, k):
                r0, rtn = starts[n], schedule[n]
                t = xvpool.tile([128, rtn], msd, tag=f"{pool_tag}{k}")
                nc.sync.dma_start(
                    t[:], dram[k * 128 : (k + 1) * 128, r0 : r0 + rtn]
                )
                return t

            def load_xv(n):
                xt = [load_one("x", x_d, n, k) for k in range(NK)]
                vt = [load_one("v", v_d, n, k) for k in range(NK)]
                return xt, vt

            # Startup order: each DMA lands exactly when its first consumer
            # needs it — x(k0), Wq(k0) lets the very first matmul fire after
            # ~0.5MB; then x(k1), Wq(k1) for the accumulate, bias for the
            # ACT, then the v-side the same way.
            NQ = 4
            qw = FH // NQ  # 512 columns per piece
            wq_sb = [[None] * NQ for _ in range(NK)]
            wv_sb = [[None] * NQ for _ in range(NK)]

            def load_w(dst, dram, q, k, nm):
                qs = slice(q * qw, (q + 1) * qw)
                t = wpool.tile([128, qw], msd, tag=f"{nm}{k}q{q}")
                nc.sync.dma_start(t[:], dram[k * 128 : (k + 1) * 128, qs])
                dst[k][q] = t

            x0 = []
            for k in range(NK):
                x0.append(load_one("x", x_d, 0, k))
                load_w(wq_sb, wq_d, 0, k, "wq")
            bq_sb = wpool.tile([128, NM], f32, tag="bq")
            nc.sync.dma_start(bq_sb[:], bq_d[:, :])
            v0 = []
            for k in range(NK):
                v0.append(load_one("v", v_d, 0, k))
                load_w(wv_sb, wv_d, 0, k, "wv")
            bv_sb = wpool.tile([128, NM], f32, tag="bv")
            nc.sync.dma_start(bv_sb[:], bv_d[:, :])
            xv0 = (x0, v0)
            for q in range(1, NQ):
                for k in range(NK):
                    load_w(wq_sb, wq_d, q, k, "wq")
                for k in range(NK):
                    load_w(wv_sb, wv_d, q, k, "wv")
            bc_sb = wpool.tile([128, 2 * D], f32, tag="bc")
            nc.sync.dma_start(bc_sb[:], bc_d[:, :])

            mpq = qw // 128  # m-slices per piece

            def wq_ap(k, m):
                return wq_sb[k][m // mpq][:, (m % mpq) * 128 : (m % mpq + 1) * 128]

            def wv_ap(k, m):
                return wv_sb[k][m // mpq][:, (m % mpq) * 128 : (m % mpq + 1) * 128]
            wc_sb = []
            for m in range(NM):
                t = wpool.tile([128, D], pt_dt, tag=f"wc{m}")
                nc.sync.dma_start(t[:], wc_d[m * 128 : (m + 1) * 128, :])
                wc_sb.append(t)

            for rep in range(reps):
                for n in range(len(schedule)):
                    r0, rtn = starts[n], schedule[n]
                    if rep == 0 and n == 0:
                        xt, vt = xv0
                    else:
                        xt, vt = load_xv(n)

                    pt = ppool.tile([128, NM * rtn], pt_dt, tag="pt")
                    for m in range(NM):
                        qp = qpsum.tile([128, rtn], f32, tag="qp")
                        for hs in mm_chunks(rtn):
                            for k in range(NK):
                                nc.tensor.matmul(
                                    qp[:, hs],
                                    wq_ap(k, m),
                                    xt[k][:, hs],
                                    start=(k == 0),
                                    stop=(k == NK - 1),
                                )
                        vp = vpsum.tile([128, rtn], f32, tag="vp")
                        for hs in mm_chunks(rtn):
                            for k in range(NK):
                                nc.tensor.matmul(
                                    vp[:, hs],
                                    wv_ap(k, m),
                                    vt[k][:, hs],
                                    start=(k == 0),
                                    stop=(k == NK - 1),
                                )
                        if probe == "pe_only":
                            continue
                        if probe == "pe_cp":
                            # timing probe: pt via cheap DVE copy, no ACT
                            nc.vector.tensor_copy(
                                pt[:, m * rtn : (m + 1) * rtn], qp[:]
                            )
                            continue
                        if probe == "no_act":
                            # timing probe: multiply straight from both PSUMs
                            nc.vector.tensor_mul(
                                pt[:, m * rtn : (m + 1) * rtn], qp[:], vp[:]
                            )
                            continue
                        qb = qvpool.tile([128, rtn], ew_dt, tag="qb")
                        if 2 * m + 1 < dve_bias_per_tile:
                            nc.vector.tensor_scalar_add(
                                qb[:], qp[:], bq_sb[:, m : m + 1]
                            )
                        else:
                            nc.scalar.activation(
                                qb[:], qp[:], Act_Id, bias=bq_sb[:, m : m + 1]
                            )
                        vb = qvpool.tile([128, rtn], ew_dt, tag="vb")
                        if 2 * m < dve_bias_per_tile:
                            nc.vector.tensor_scalar_add(
                                vb[:], vp[:], bv_sb[:, m : m + 1]
                            )
                        else:
                            nc.scalar.activation(
                                vb[:], vp[:], Act_Id, bias=bv_sb[:, m : m + 1]
                            )
                        mul_eng = nc.gpsimd if m < mul_on_pool else nc.vector
                        mul_eng.tensor_mul(
                            pt[:, m * rtn : (m + 1) * rtn], qb[:], vb[:]
                        )

                    if probe == "pe_only":
                        continue  # q/v matmuls only
                    if merge_sp:
                        for sp in range(rtn // 256):
                            # two 128-row c-proj groups share one PSUM bank;
                            # one bias-add + one (rearranged) store for both
                            op = opsum.tile([128, 2 * D], f32, tag="op")
                            for half in range(2):
                                s = 2 * sp + half
                                oslice = slice(half * D, (half + 1) * D)
                                for m in range(NM):
                                    c0 = m * rtn + s * 128
                                    nc.tensor.matmul(
                                        op[:, oslice],
                                        pt[:, c0 : c0 + 128],
                                        wc_sb[m][:],
                                        start=(m == 0),
                                        stop=(m == NM - 1),
                                    )
                            ot = opool.tile([128, 2 * D], f32, tag="ot")
                            nc.vector.tensor_add(ot[:], op[:], bc_sb[:])
                            dst = out_d[
                                r0 + sp * 256 : r0 + (sp + 1) * 256, :
                            ].rearrange("(two p) c -> p two c", two=2)
                            nc.sync.dma_start(
                                dst,
                                ot[:].rearrange("p (two c) -> p two c", two=2),
                            )
                    else:
                        for s in range(rtn // 128):
                            op = opsum.tile([128, D], f32, tag="op")
                            for m in range(NM):
                                c0 = m * rtn + s * 128
                                nc.tensor.matmul(
                                    op[:],
                                    pt[:, c0 : c0 + 128],
                                    wc_sb[m][:],
                                    start=(m == 0),
                                    stop=(m == NM - 1),
                                )
                            ot = opool.tile([128, D], f32, tag="ot")
                            nc.vector.tensor_add(ot[:], op[:], bc_sb[:, :D])
                            nc.sync.dma_start(
                                out_d[r0 + s * 128 : r0 + (s + 1) * 128, :],
                                ot[:],
                            )

    if compile:
        nc.compile()
    return nc


def prep_in_maps(query_key_input, value, Wq, bq, Wv, bv, Wc, bc, qvp_bf16=False):
    """Host-side shard + layout prep. Returns list of 8 per-core input dicts."""
    if qvp_bf16:
        import ml_dtypes

        wc_np = ml_dtypes.bfloat16
    else:
        wc_np = np.float32
    x = np.asarray(query_key_input, dtype=np.float32)
    v = np.asarray(value, dtype=np.float32)
    shared = {
        "wqT": np.ascontiguousarray(np.asarray(Wq, np.float32).T),
        "wvT": np.ascontiguousarray(np.asarray(Wv, np.float32).T),
        "wcT": np.ascontiguousarray(np.asarray(Wc, np.float32).T.astype(wc_np)),
        "bq2": np.ascontiguousarray(np.asarray(bq, np.float32).reshape(NM, 128).T),
        "bv2": np.ascontiguousarray(np.asarray(bv, np.float32).reshape(NM, 128).T),
        "bcb": np.ascontiguousarray(
            np.broadcast_to(
                np.tile(np.asarray(bc, np.float32), 2), (128, 2 * D)
            )
        ),
    }
    in_maps = []
    for c in range(N_CORES):
        rows = slice(c * R, (c + 1) * R)
        m = dict(shared)
        m["xT"] = np.ascontiguousarray(x[rows].T)
        m["vT"] = np.ascontiguousarray(v[rows].T)
        in_maps.append(m)
    return in_maps


def run_program(nc, in_maps):
    from concourse import bass_utils

    res = bass_utils.run_bass_kernel_spmd(nc, in_maps, core_ids=list(range(N_CORES)))
    return res


def kernel(query_key_input, value, Wq, bq, Wk, bk, Wv, bv, Wc, bc):
    if "nc" not in _CACHE:
        _CACHE["nc"] = build_program(reps=1)
    nc = _CACHE["nc"]
    in_maps = prep_in_maps(query_key_input, value, Wq, bq, Wv, bv, Wc, bc)
    res = run_program(nc, in_maps)
    out = np.concatenate([res.results[c]["out"] for c in range(N_CORES)], axis=0)
    return out


# revision 40
# speedup vs baseline: 1.1538x; 1.0400x over previous
"""Trainium2 Bass kernel for nn_MultiHeadAttention_76510547410991.

The reference's reshapes apply identically to both factors of the
elementwise product, so they cancel and the computation is exactly:
    out = ((x @ Wq.T + bq) * (value @ Wv.T + bv)) @ Wc.T + bc

Sharding: rows (S=32768) split across 8 cores, 4096 rows each; weights
replicated.  All activations are kept in the transposed (feature-major)
domain on-chip so that neither the Q/V projections nor the final
C-projection need any on-device transposes; the host pre-transposes the
inputs (cheap numpy copies, outside the device clock).

All matmuls run in float32r (TF32-like PE fast path, 4x the fp32 rate;
measured end-to-end relative error ~2.5e-4 vs fp64).

Per-core dataflow, row-tile RT=512:
  xT,vT [256, 4096]   (host-transposed shards)
  for each row-tile n:
    for m in 16 feature slices of 2048:
      qp[128,512](PSUM)  = WqT_k-slices.T @ xT_k        (2 matmuls, f32r)
      vp[128,512](PSUM)  = WvT_k-slices.T @ vT_k
      qb = ACT(qp + bq_m)  (PSUM->SBUF, per-partition bias fused)
      vb = ACT(vp + bv_m)   (even m: DVE tensor_scalar instead — ACT and
                             PE are co-saturated; this offloads 25% of
                             ACT to DVE's slack, -19us measured)
      pT_m = DVE qb*vb     (SBUF, f32r)
    for s in 4 row-subtiles of 128:
      op[128,256](PSUM) = sum_m pT_m_s.T @ WcT_m        (16 matmuls, f32r)
      out_s = DVE (op + bc)                              (bias broadcast)
      DMA out_s -> out[rows, 256]
Startup DMAs are ordered x0/bq/Wq-piece/v0/bv/Wv-piece/... so the first
matmul fires after ~1MB of DMA instead of after all 6MB of weights.
PSUM banks are split 3 (q) / 2 (v) / 3 (c-proj out) — HW-A/B-tuned.

Measured (8 axon-tunneled trn2 cores, steady state via in-NEFF-reps
A/B deltas): ~145-157 us per invocation, against a ~132 us pure-matmul
floor at the observed PE rate (512 q/v matmuls alone measure ~88 us;
f32r runs ~173 ns per 512-row matmul on HW).  Engine budget per core:
PE ~132 us, ACT 256 bias ops ~130-158 us, DVE ~89 us, DMA 18 MB ~53 us.
"""

import numpy as np

try:
    import concourse.bacc  # noqa: F401
except ImportError:  # fresh environment without the default sys.path setup
    import sys

    for p in ("/opt/trn_rl_repo", "/opt/pypackages"):
        if p not in sys.path:
            sys.path.insert(0, p)

H = 8
F = 256
S = 32768
FH = F * H  # 2048
D = F  # output features 256
N_CORES = 8
R = S // N_CORES  # 4096 rows per core
RT = 512  # rows per row-tile (fp32 moving-operand max)
NT = R // RT  # 8 row tiles per core
NM = FH // 128  # 16 feature slices
NK = F // 128  # 2 contraction slices for q/v proj

_CACHE = {}


def build_program(
    reps=1,
    mm_mode="f32r",
    qpsum_bufs=3,
    vpsum_bufs=2,
    opsum_bufs=3,
    qv_bufs=4,
    xv_bufs=3,
    pt_bufs=2,
    o_bufs=4,
    rt=RT,
    dve_bias_per_tile=0,  # 0..2*NM: how many of the bias ops go to DVE
    alt_bias=True,  # v-bias of even m on DVE (keeps DVE chain at TSP+mul)
    mul_on_pool=0,  # 0..NM: how many of the per-m muls go to GPSIMD
    merge_sp=False,  # merge c-proj 128-row subtile pairs into one PSUM bank
    probe=None,  # "pe_only" | "pe_cp" | "no_act" — timing-only diagnostics
    taper=False,  # 256-row first/last tiles (sim: net loss, keep off)
    qvp_bf16=False,  # qb/vb/pt (and Wc) in bf16: 2x DVE mul, bf16 c-proj
    compile=True,
):
    """Build + compile the per-core Bass program (identical on all cores)."""
    import concourse.bacc as bacc
    import concourse.mybir as mybir
    import concourse.tile as tile

    f32 = mybir.dt.float32
    if mm_mode == "f32r":
        msd = mybir.dt.float32r  # storage dtype for matmul operands
    elif mm_mode == "f32":
        msd = f32
    else:
        raise ValueError(mm_mode)
    bf16 = mybir.dt.bfloat16
    ew_dt = bf16 if qvp_bf16 else f32  # qb/vb dtype
    pt_dt = bf16 if qvp_bf16 else msd  # pt / wc dtype (c-proj operands)

    nc = bacc.Bacc(
        "TRN2",
        target_bir_lowering=False,
        debug=False,
        enable_asserts=False,
        num_devices=N_CORES,
    )

    x_d = nc.dram_tensor("xT", [F, R], msd, kind="ExternalInput").ap()
    v_d = nc.dram_tensor("vT", [F, R], msd, kind="ExternalInput").ap()
    wq_d = nc.dram_tensor("wqT", [F, FH], msd, kind="ExternalInput").ap()
    wv_d = nc.dram_tensor("wvT", [F, FH], msd, kind="ExternalInput").ap()
    wc_d = nc.dram_tensor("wcT", [FH, D], pt_dt, kind="ExternalInput").ap()
    bq_d = nc.dram_tensor("bq2", [128, NM], f32, kind="ExternalInput").ap()
    bv_d = nc.dram_tensor("bv2", [128, NM], f32, kind="ExternalInput").ap()
    bc_d = nc.dram_tensor("bcb", [128, 2 * D], f32, kind="ExternalInput").ap()
    out_d = nc.dram_tensor("out", [R, D], f32, kind="ExternalOutput").ap()

    Act_Id = mybir.ActivationFunctionType.Identity

    if taper:
        # small first tile -> first matmuls fire after ~0.7MB of DMA;
        # small last tile -> shorter final dependency chain.
        schedule = [256] + [rt] * ((R - 512) // rt) + [256]
    else:
        schedule = [rt] * (R // rt)
    assert sum(schedule) == R
    starts = [sum(schedule[:i]) for i in range(len(schedule))]

    def mm_chunks(rtn):
        # moving-dim chunks of <=512 (f32r needs >=256 for full rate)
        return [slice(h, min(h + 512, rtn)) for h in range(0, rtn, 512)]

    with tile.TileContext(nc) as tc:
        with (
            tc.tile_pool(name="w", bufs=1) as wpool,
            tc.tile_pool(name="xv", bufs=xv_bufs) as xvpool,
            tc.tile_pool(name="qv", bufs=qv_bufs) as qvpool,
            tc.tile_pool(name="p", bufs=pt_bufs) as ppool,
            tc.tile_pool(name="o", bufs=o_bufs) as opool,
            tc.tile_pool(name="qpsum", bufs=qpsum_bufs, space="PSUM") as qpsum,
            tc.tile_pool(
                name="vpsum",
                bufs=vpsum_bufs if vpsum_bufs is not None else qpsum_bufs,
                space="PSUM",
            ) as vpsum,
            tc.tile_pool(name="opsum", bufs=opsum_bufs, space="PSUM") as opsum,
        ):
            def load_one(pool_tag, dram, n, k):
                r0, rtn = starts[n], schedule[n]
                t = xvpool.tile([128, rtn], msd, tag=f"{pool_tag}{k}")
                nc.sync.dma_start(
                    t[:], dram[k * 128 : (k + 1) * 128, r0 : r0 + rtn]
                )
                return t

            def load_xv(n):
                xt = [load_one("x", x_d, n, k) for k in range(NK)]
                vt = [load_one("v", v_d, n, k) for k in range(NK)]
                return xt, vt

            # Startup order: each DMA lands exactly when its first consumer
            # needs it — x(k0), Wq(k0) lets the very first matmul fire after
            # ~0.5MB; then x(k1), Wq(k1) for the accumulate, bias for the
            # ACT, then the v-side the same way.
            NQ = 4
            qw = FH // NQ  # 512 columns per piece
            wq_sb = [[None] * NQ for _ in range(NK)]
            wv_sb = [[None] * NQ for _ in range(NK)]

            def load_w(dst, dram, q, k, nm):
                qs = slice(q * qw, (q + 1) * qw)
                t = wpool.tile([128, qw], msd, tag=f"{nm}{k}q{q}")
                nc.sync.dma_start(t[:], dram[k * 128 : (k + 1) * 128, qs])
                dst[k][q] = t

            x0 = []
            for k in range(NK):
                x0.append(load_one("x", x_d, 0, k))
                load_w(wq_sb, wq_d, 0, k, "wq")
            bq_sb = wpool.tile([128, NM], f32, tag="bq")
            nc.sync.dma_start(bq_sb[:], bq_d[:, :])
            v0 = []
            for k in range(NK):
                v0.append(load_one("v", v_d, 0, k))
                load_w(wv_sb, wv_d, 0, k, "wv")
            bv_sb = wpool.tile([128, NM], f32, tag="bv")
            nc.sync.dma_start(bv_sb[:], bv_d[:, :])
            xv0 = (x0, v0)
            for q in range(1, NQ):
                for k in range(NK):
                    load_w(wq_sb, wq_d, q, k, "wq")
                for k in range(NK):
                    load_w(wv_sb, wv_d, q, k, "wv")
            bc_sb = wpool.tile([128, 2 * D], f32, tag="bc")
            nc.sync.dma_start(bc_sb[:], bc_d[:, :])

            mpq = qw // 128  # m-slices per piece

            def wq_ap(k, m):
                return wq_sb[k][m // mpq][:, (m % mpq) * 128 : (m % mpq + 1) * 128]

            def wv_ap(k, m):
                return wv_sb[k][m // mpq][:, (m % mpq) * 128 : (m % mpq + 1) * 128]
            wc_sb = []
            for m in range(NM):
                t = wpool.tile([128, D], pt_dt, tag=f"wc{m}")
                nc.sync.dma_start(t[:], wc_d[m * 128 : (m + 1) * 128, :])
                wc_sb.append(t)

            for rep in range(reps):
                for n in range(len(schedule)):
                    r0, rtn = starts[n], schedule[n]
                    if rep == 0 and n == 0:
                        xt, vt = xv0
                    else:
                        xt, vt = load_xv(n)

                    pt = ppool.tile([128, NM * rtn], pt_dt, tag="pt")
                    for m in range(NM):
                        qp = qpsum.tile([128, rtn], f32, tag="qp")
                        for hs in mm_chunks(rtn):
                            for k in range(NK):
                                nc.tensor.matmul(
                                    qp[:, hs],
                                    wq_ap(k, m),
                                    xt[k][:, hs],
                                    start=(k == 0),
                                    stop=(k == NK - 1),
                                )
                        vp = vpsum.tile([128, rtn], f32, tag="vp")
                        for hs in mm_chunks(rtn):
                            for k in range(NK):
                                nc.tensor.matmul(
                                    vp[:, hs],
                                    wv_ap(k, m),
                                    vt[k][:, hs],
                                    start=(k == 0),
                                    stop=(k == NK - 1),
                                )
                        if probe == "pe_only":
                            continue
                        if probe == "pe_cp":
                            # timing probe: pt via cheap DVE copy, no ACT
                            nc.vector.tensor_copy(
                                pt[:, m * rtn : (m + 1) * rtn], qp[:]
                            )
                            continue
                        if probe == "no_act":
                            # timing probe: multiply straight from both PSUMs
                            nc.vector.tensor_mul(
                                pt[:, m * rtn : (m + 1) * rtn], qp[:], vp[:]
                            )
                            continue
                        qb = qvpool.tile([128, rtn], ew_dt, tag="qb")
                        if 2 * m + 1 < dve_bias_per_tile:
                            nc.vector.tensor_scalar_add(
                                qb[:], qp[:], bq_sb[:, m : m + 1]
                            )
                        else:
                            nc.scalar.activation(
                                qb[:], qp[:], Act_Id, bias=bq_sb[:, m : m + 1]
                            )
                        vb = qvpool.tile([128, rtn], ew_dt, tag="vb")
                        if (alt_bias and m % 2 == 0) or 2 * m < dve_bias_per_tile:
                            nc.vector.tensor_scalar_add(
                                vb[:], vp[:], bv_sb[:, m : m + 1]
                            )
                        else:
                            nc.scalar.activation(
                                vb[:], vp[:], Act_Id, bias=bv_sb[:, m : m + 1]
                            )
                        mul_eng = nc.gpsimd if m < mul_on_pool else nc.vector
                        mul_eng.tensor_mul(
                            pt[:, m * rtn : (m + 1) * rtn], qb[:], vb[:]
                        )

                    if probe == "pe_only":
                        continue  # q/v matmuls only
                    if merge_sp:
                        for sp in range(rtn // 256):
                            # two 128-row c-proj groups share one PSUM bank;
                            # one bias-add + one (rearranged) store for both
                            op = opsum.tile([128, 2 * D], f32, tag="op")
                            for half in range(2):
                                s = 2 * sp + half
                                oslice = slice(half * D, (half + 1) * D)
                                for m in range(NM):
                                    c0 = m * rtn + s * 128
                                    nc.tensor.matmul(
                                        op[:, oslice],
                                        pt[:, c0 : c0 + 128],
                                        wc_sb[m][:],
                                        start=(m == 0),
                                        stop=(m == NM - 1),
                                    )
                            ot = opool.tile([128, 2 * D], f32, tag="ot")
                            nc.vector.tensor_add(ot[:], op[:], bc_sb[:])
                            dst = out_d[
                                r0 + sp * 256 : r0 + (sp + 1) * 256, :
                            ].rearrange("(two p) c -> p two c", two=2)
                            nc.sync.dma_start(
                                dst,
                                ot[:].rearrange("p (two c) -> p two c", two=2),
                            )
                    else:
                        for s in range(rtn // 128):
                            op = opsum.tile([128, D], f32, tag="op")
                            for m in range(NM):
                                c0 = m * rtn + s * 128
                                nc.tensor.matmul(
                                    op[:],
                                    pt[:, c0 : c0 + 128],
                                    wc_sb[m][:],
                                    start=(m == 0),
                                    stop=(m == NM - 1),
                                )
                            ot = opool.tile([128, D], f32, tag="ot")
                            nc.vector.tensor_add(ot[:], op[:], bc_sb[:, :D])
                            nc.sync.dma_start(
                                out_d[r0 + s * 128 : r0 + (s + 1) * 128, :],
                                ot[:],
                            )

    if compile:
        nc.compile()
    return nc


def prep_in_maps(query_key_input, value, Wq, bq, Wv, bv, Wc, bc, qvp_bf16=False):
    """Host-side shard + layout prep. Returns list of 8 per-core input dicts."""
    if qvp_bf16:
        import ml_dtypes

        wc_np = ml_dtypes.bfloat16
    else:
        wc_np = np.float32
    x = np.asarray(query_key_input, dtype=np.float32)
    v = np.asarray(value, dtype=np.float32)
    shared = {
        "wqT": np.ascontiguousarray(np.asarray(Wq, np.float32).T),
        "wvT": np.ascontiguousarray(np.asarray(Wv, np.float32).T),
        "wcT": np.ascontiguousarray(np.asarray(Wc, np.float32).T.astype(wc_np)),
        "bq2": np.ascontiguousarray(np.asarray(bq, np.float32).reshape(NM, 128).T),
        "bv2": np.ascontiguousarray(np.asarray(bv, np.float32).reshape(NM, 128).T),
        "bcb": np.ascontiguousarray(
            np.broadcast_to(
                np.tile(np.asarray(bc, np.float32), 2), (128, 2 * D)
            )
        ),
    }
    in_maps = []
    for c in range(N_CORES):
        rows = slice(c * R, (c + 1) * R)
        m = dict(shared)
        m["xT"] = np.ascontiguousarray(x[rows].T)
        m["vT"] = np.ascontiguousarray(v[rows].T)
        in_maps.append(m)
    return in_maps


def run_program(nc, in_maps):
    from concourse import bass_utils

    res = bass_utils.run_bass_kernel_spmd(nc, in_maps, core_ids=list(range(N_CORES)))
    return res


class _Runner:
    """Cached PJRT executable for the compiled program: repeat kernel()
    calls skip retracing/recompiling (mirrors bass2jax.run_bass_via_pjrt)."""

    def __init__(self, nc):
        import jax
        from jax.sharding import Mesh, NamedSharding, PartitionSpec

        import concourse.mybir as mybir
        from concourse.bass2jax import _bass_exec_p, install_neuronx_cc_hook

        try:
            from jax.experimental.shard_map import shard_map
        except ImportError:
            from jax.shard_map import shard_map

        install_neuronx_cc_hook()
        assert nc.partition_id_tensor is None and nc.dbg_addr is None
        self.jax = jax
        in_names = []
        out_names = []
        out_avals = []
        self.out_shapes = {}
        for alloc in nc.m.functions[0].allocations:
            if not isinstance(alloc, mybir.MemoryLocationSet):
                continue
            name = alloc.memorylocations[0].name
            if alloc.kind == "ExternalInput":
                in_names.append(name)
            elif alloc.kind == "ExternalOutput":
                shape = tuple(alloc.tensor_shape)
                dtype = mybir.dt.np(alloc.dtype)
                out_names.append(name)
                out_avals.append(jax.core.ShapedArray(shape, dtype))
                self.out_shapes[name] = (shape, dtype)
        self.in_names = in_names
        self.out_names = out_names
        n_params = len(in_names)
        all_in = list(in_names) + list(out_names)
        donate = tuple(range(n_params, n_params + len(out_names)))

        def _body(*args):
            return tuple(
                _bass_exec_p.bind(
                    *args,
                    out_avals=tuple(out_avals),
                    in_names=tuple(all_in),
                    out_names=tuple(out_names),
                    lowering_input_output_aliases=(),
                    sim_require_finite=True,
                    sim_require_nnan=True,
                    nc=nc,
                )
            )

        devices = jax.devices()[:N_CORES]
        mesh = Mesh(np.asarray(devices), ("core",))
        specs = (PartitionSpec("core"),) * (n_params + len(out_names))
        self.sharding = NamedSharding(mesh, PartitionSpec("core"))
        self.fn = jax.jit(
            shard_map(
                _body,
                mesh=mesh,
                in_specs=specs,
                out_specs=(PartitionSpec("core"),) * len(out_names),
                check_rep=False,
            ),
            donate_argnums=donate,
            keep_unused=True,
        )

    def __call__(self, in_maps):
        jax = self.jax
        ins = [
            jax.device_put(
                np.concatenate([np.asarray(m[n]) for m in in_maps], axis=0),
                self.sharding,
            )
            for n in self.in_names
        ]
        zouts = [
            jax.device_put(
                np.zeros((N_CORES * s[0], *s[1:]), d), self.sharding
            )
            for s, d in (self.out_shapes[n] for n in self.out_names)
        ]
        outs = self.fn(*ins, *zouts)
        res = []
        for c in range(N_CORES):
            d = {}
            for i, n in enumerate(self.out_names):
                s, _ = self.out_shapes[n]
                d[n] = np.asarray(outs[i]).reshape(N_CORES, *s)[c]
            res.append(d)
        return res


def kernel(query_key_input, value, Wq, bq, Wk, bk, Wv, bv, Wc, bc):
    in_maps = prep_in_maps(query_key_input, value, Wq, bq, Wv, bv, Wc, bc)
    if "nc" not in _CACHE:
        _CACHE["nc"] = build_program(reps=1)
    nc = _CACHE["nc"]
    try:
        if "runner" not in _CACHE:
            _CACHE["runner"] = _Runner(nc)
        results = _CACHE["runner"](in_maps)
    except Exception:
        _CACHE.pop("runner", None)
        results = run_program(nc, in_maps).results
    out = np.concatenate([results[c]["out"] for c in range(N_CORES)], axis=0)
    return out
